# revision 12
# baseline (speedup 1.0000x reference)
"""Positional-embedding lookup kernel for TRN2 (8 NeuronCores).

out[b, l, :] = emb_table[input_ids[b, l], :] + pe[l, :]

Sharding: data-parallel over batch (batch == n_cores == 8). Core b handles
batch row b.

Per 128-token tile, fully on-device:
  1. PE engine broadcasts the per-tile angle offsets (ones-matmul, K=1)
     into PSUM.
  2. DVE adds the per-partition base angles and reduces mod 2pi.
  3. ACT computes Sin(m - pi)  (== sin/cos of the true angle; host
     pre-wrapped the angle bases so m - pi is congruent to the angle).
  4. GPSIMD indirect-DMA gathers the 128 embedding rows, accumulating
     onto the PE tile with the SDMA inline CCE adder.
  5. The finished tile is stored to DRAM.

The sinusoidal table is therefore never read from HBM: per-core traffic is
16 MiB gather + 16 MiB store (+ tiny angle tables).

Raw Bass (not Tile): the per-instruction sync-wait slots on this compiler
are too few for this pipeline, so semaphores are managed manually with
standalone wait_ge instructions. DMA completion sems are per-buffer-slot:
same-slot DMAs are serialized by the pipeline, which makes per-slot
counting sound despite cross-SDMA-engine completion skew.
"""

import contextlib
import math

import numpy as np

import concourse.bass as bass
import concourse.mybir as mybir
from concourse.bass_utils import run_bass_kernel_spmd

VOCAB = 50257
D = 1024
BATCH = 8
SEQ = 4096
P = 128
N_TILES = SEQ // P  # 32
N_CORES = 8
B = 8  # tok ring buffers
BA = 4  # angle ring buffers
BP = 2  # psum ring buffers

TWO_PI = float(np.float32(2 * math.pi))

_CACHED = {}


def _angle_tables():
    """Host-precomputed, pre-wrapped angle tables (f64 -> f32).

    base2[p, 2k]   = (p*f_k + pi) mod 2pi          (sin column)
    base2[p, 2k+1] = (p*f_k + pi/2 + pi) mod 2pi   (cos column)
    step2[i, c]    = (128*i*f_(c//2)) mod 2pi

    On device: m = (base2 + step2[i]) mod 2pi in [0, 2pi), and
    Sin(m - pi) == sin(angle + [0 or pi/2]) which is the interleaved
    sin/cos positional encoding.
    """
    k = np.arange(D // 2, dtype=np.float64)
    f = np.power(10000.0, -2.0 * k / D)  # [512]
    p = np.arange(P, dtype=np.float64)[:, None]
    i = np.arange(N_TILES, dtype=np.float64)[:, None]

    base_sin = np.mod(p * f + math.pi, 2 * math.pi)
    base_cos = np.mod(p * f + math.pi / 2 + math.pi, 2 * math.pi)
    base2 = np.empty((P, D), np.float32)
    base2[:, 0::2] = base_sin.astype(np.float32)
    base2[:, 1::2] = base_cos.astype(np.float32)

    step = np.mod(128.0 * i * f, 2 * math.pi)  # [32, 512]
    step2 = np.empty((N_TILES, D), np.float32)
    step2[:, 0::2] = step.astype(np.float32)
    step2[:, 1::2] = step.astype(np.float32)
    return base2, step2


def _build_nc():
    nc = bass.Bass("TRN2")
    ids_t = nc.dram_tensor("ids", [P, N_TILES], mybir.dt.int32, kind="ExternalInput")
    table_t = nc.dram_tensor(
        "table", [VOCAB, D], mybir.dt.float32, kind="ExternalInput"
    )
    base_t = nc.dram_tensor("base2", [P, D], mybir.dt.float32, kind="ExternalInput")
    step_t = nc.dram_tensor(
        "step2", [1, N_TILES * D], mybir.dt.float32, kind="ExternalInput"
    )
    ones_t = nc.dram_tensor("ones", [1, P], mybir.dt.float32, kind="ExternalInput")
    bias_t = nc.dram_tensor("negpi", [P, 1], mybir.dt.float32, kind="ExternalInput")
    out_t = nc.dram_tensor("out", [SEQ, D], mybir.dt.float32, kind="ExternalOutput")

    with contextlib.ExitStack() as ctx:
        ids_sb = ctx.enter_context(
            nc.sbuf_tensor("ids_sb", [P, N_TILES], mybir.dt.int32)
        )
        base_sb = ctx.enter_context(nc.sbuf_tensor("base_sb", [P, D], mybir.dt.float32))
        step_sb = ctx.enter_context(
            nc.sbuf_tensor("step_sb", [1, N_TILES * D], mybir.dt.float32)
        )
        ones_sb = ctx.enter_context(nc.sbuf_tensor("ones_sb", [1, P], mybir.dt.float32))
        bias_sb = ctx.enter_context(nc.sbuf_tensor("bias_sb", [P, 1], mybir.dt.float32))
        ang_sb = ctx.enter_context(
            nc.sbuf_tensor("ang_sb", [P, BA * D], mybir.dt.float32)
        )
        scr_sb = ctx.enter_context(nc.sbuf_tensor("scr_sb", [P, D], mybir.dt.float32))
        tok_sb = ctx.enter_context(
            nc.sbuf_tensor("tok_sb", [P, B * D], mybir.dt.float32)
        )
        psum = ctx.enter_context(
            nc.psum_tensor("psum_step", [P, BP * D], mybir.dt.float32)
        )

        s_ids = ctx.enter_context(nc.semaphore("s_ids"))
        s_base = ctx.enter_context(nc.semaphore("s_base"))
        s_step = ctx.enter_context(nc.semaphore("s_step"))
        s_ones = ctx.enter_context(nc.semaphore("s_ones"))
        s_bias = ctx.enter_context(nc.semaphore("s_bias"))
        s_mm = ctx.enter_context(nc.semaphore("s_mm"))  # PE matmuls (in-order)
        s_dve = ctx.enter_context(nc.semaphore("s_dve"))  # DVE ops (in-order)
        s_act = ctx.enter_context(nc.semaphore("s_act"))  # ACT sins (in-order)
        # per-slot DMA completion sems (SDMA engines complete with skew)
        s_g = [ctx.enter_context(nc.semaphore(f"s_g{j}")) for j in range(B)]
        s_st = [ctx.enter_context(nc.semaphore(f"s_st{j}")) for j in range(B)]
        block = ctx.enter_context(nc.Block())

        @block.sync
        def _(sync):
            sync.dma_start(ids_sb[:], ids_t[:]).then_inc(s_ids, 16)
            sync.dma_start(base_sb[:], base_t[:]).then_inc(s_base, 16)
            sync.dma_start(step_sb[:], step_t[:]).then_inc(s_step, 16)
            sync.dma_start(ones_sb[:], ones_t[:]).then_inc(s_ones, 16)
            sync.dma_start(bias_sb[:], bias_t[:]).then_inc(s_bias, 16)
            for i in range(N_TILES):
                j = i % B
                sync.wait_ge(s_g[j], 16 * (i // B + 1))
                sync.dma_start(
                    out_t[i * P : (i + 1) * P, :], tok_sb[:, j * D : (j + 1) * D]
                ).then_inc(s_st[j], 16)
            for j in range(B):
                sync.wait_ge(s_st[j], 16 * (N_TILES // B))

        @block.tensor
        def _(t):
            t.wait_ge(s_ones, 16)
            t.wait_ge(s_step, 16)
            for i in range(N_TILES):
                jp = i % BP
                if i >= BP:
                    # psum slot reuse: DVE sum of tile i-BP must have read it
                    t.wait_ge(s_dve, 3 * (i - BP) + 1)
                for h in range(2):  # two 512-wide matmuls (one PSUM bank each)
                    mm = t.matmul(
                        psum[:, jp * D + h * 512 : jp * D + (h + 1) * 512],
                        ones_sb[:, :],
                        step_sb[0:1, i * D + h * 512 : i * D + (h + 1) * 512],
                        start=True,
                        stop=True,
                    )
                mm.then_inc(s_mm, 1)

        @block.vector
        def _(v):
            v.wait_ge(s_base, 16)
            for i in range(N_TILES):
                ja = i % BA
                jp = i % BP
                asl = slice(ja * D, (ja + 1) * D)
                v.wait_ge(s_mm, i + 1)
                if i >= BA:
                    # ang slot reuse: ACT of tile i-BA must have read it
                    v.wait_ge(s_act, i - BA + 1)
                v.tensor_tensor(
                    out=ang_sb[:, asl],
                    in0=base_sb[:, :],
                    in1=psum[:, jp * D : (jp + 1) * D],
                    op=mybir.AluOpType.add,
                ).then_inc(s_dve, 1)
                # scr = 2pi where ang >= 2pi else 0; ang -= scr  (mod 2pi)
                v.tensor_scalar(
                    out=scr_sb[:, :],
                    in0=ang_sb[:, asl],
                    scalar1=TWO_PI,
                    scalar2=TWO_PI,
                    op0=mybir.AluOpType.is_ge,
                    op1=mybir.AluOpType.mult,
                ).then_inc(s_dve, 1)
                v.tensor_tensor(
                    out=ang_sb[:, asl],
                    in0=ang_sb[:, asl],
                    in1=scr_sb[:, :],
                    op=mybir.AluOpType.subtract,
                ).then_inc(s_dve, 1)

        @block.scalar
        def _(sc):
            sc.wait_ge(s_bias, 16)
            for i in range(N_TILES):
                ja = i % BA
                j = i % B
                if i >= B:
                    # tok slot reuse: store of tile i-B must be done
                    sc.wait_ge(s_st[j], 16 * (i // B))
                sc.wait_ge(s_dve, 3 * i + 3)
                sc.activation(
                    tok_sb[:, j * D : (j + 1) * D],
                    ang_sb[:, ja * D : (ja + 1) * D],
                    mybir.ActivationFunctionType.Sin,
                    bias=bias_sb[:, :],
                ).then_inc(s_act, 1)

        @block.gpsimd
        def _(g):
            g.wait_ge(s_ids, 16)
            for i in range(N_TILES):
                j = i % B
                g.wait_ge(s_act, i + 1)
                g.indirect_dma_start(
                    out=tok_sb[:, j * D : (j + 1) * D],
                    out_offset=None,
                    in_=table_t[:],
                    in_offset=bass.IndirectOffsetOnAxis(
                        ap=ids_sb[:, i : i + 1], axis=0
                    ),
                    compute_op=mybir.AluOpType.add,
                ).then_inc(s_g[j], 16)

    return nc


def _get_nc():
    if "nc" not in _CACHED:
        _CACHED["nc"] = _build_nc()
    return _CACHED["nc"]


def _in_maps(input_ids, emb_table):
    ids = np.ascontiguousarray(np.asarray(input_ids)).astype(np.int32)
    table = np.ascontiguousarray(np.asarray(emb_table), dtype=np.float32)
    assert ids.shape == (BATCH, SEQ)
    assert table.shape == (VOCAB, D)

    if "tables" not in _CACHED:
        _CACHED["tables"] = _angle_tables()
    base2, step2 = _CACHED["tables"]
    ones = np.ones((1, P), np.float32)
    negpi = np.full((P, 1), -math.pi, np.float32)

    in_maps = []
    for b in range(N_CORES):
        # column i of ids_re holds tokens for positions i*128 .. i*128+127
        ids_re = np.ascontiguousarray(ids[b].reshape(N_TILES, P).T)
        in_maps.append(
            {
                "ids": ids_re,
                "table": table,
                "base2": base2,
                "step2": step2.reshape(1, -1),
                "ones": ones,
                "negpi": negpi,
            }
        )
    return in_maps


def kernel(input_ids, emb_table, encoder_seq_L):
    nc = _get_nc()
    in_maps = _in_maps(input_ids, emb_table)
    res = run_bass_kernel_spmd(nc, in_maps, core_ids=list(range(N_CORES)))
    out = np.stack([r["out"] for r in res.results], axis=0)
    return out


# revision 14
# speedup vs baseline: 1.7244x; 1.7244x over previous
"""Positional-embedding lookup kernel for TRN2 (8 NeuronCores).

out[b, l, :] = emb_table[input_ids[b, l], :] + pe[l, :]

Sharding: data-parallel over SEQUENCE. Core j handles positions
[j*512, (j+1)*512) of all 8 batch rows (4096 tokens). This keeps the
per-core gather/store traffic identical to batch sharding (16 MiB each)
but shrinks the positional-encoding read to a 2 MiB slice that stays
resident in SBUF and is reused by all 8 batch rows.

Per-core tile i (0..31): batch row i//4, position chunk i%4 within the
core's 512-position range. The 4 distinct PE chunks are loaded once.

Raw Bass (not Tile): the per-instruction sync-wait slots on this compiler
are too few for the gather pipeline, so semaphores are managed manually
with standalone wait_ge instructions. DMA completion sems are
per-buffer-slot: same-slot DMAs are serialized by the pipeline, which
makes per-slot counting sound despite cross-SDMA-engine completion skew.
"""

import contextlib

import numpy as np

import concourse.bass as bass
import concourse.mybir as mybir
from concourse.bass_utils import run_bass_kernel_spmd

VOCAB = 50257
D = 1024
BATCH = 8
SEQ = 4096
P = 128
N_CORES = 8
POS_PER_CORE = SEQ // N_CORES  # 512
CHUNKS = POS_PER_CORE // P  # 4 pe chunks per core
N_TILES = BATCH * CHUNKS  # 32 tiles of 128 tokens per core
B = 8  # tok ring buffers

_CACHED = {}


def _sinusoidal_table(seq_len: int, d_model: int) -> np.ndarray:
    i = np.arange(0, d_model // 2, dtype=np.float32)
    pos = np.arange(seq_len, dtype=np.float32)[:, None]
    div = pos / np.power(np.float32(10000.0), 2.0 * i / np.float32(d_model))
    pe = np.stack((np.sin(div), np.cos(div)), axis=2).reshape(seq_len, -1)
    return np.ascontiguousarray(pe[:, :d_model], dtype=np.float32)


def _build_nc():
    nc = bass.Bass("TRN2")
    ids_t = nc.dram_tensor("ids", [P, N_TILES], mybir.dt.int32, kind="ExternalInput")
    table_t = nc.dram_tensor(
        "table", [VOCAB, D], mybir.dt.float32, kind="ExternalInput"
    )
    pe_t = nc.dram_tensor("pe", [P, CHUNKS * D], mybir.dt.float32, kind="ExternalInput")
    out_t = nc.dram_tensor("out", [N_TILES * P, D], mybir.dt.float32, kind="ExternalOutput")

    with contextlib.ExitStack() as ctx:
        ids_sb = ctx.enter_context(
            nc.sbuf_tensor("ids_sb", [P, N_TILES], mybir.dt.int32)
        )
        pe_sb = ctx.enter_context(
            nc.sbuf_tensor("pe_sb", [P, CHUNKS * D], mybir.dt.float32)
        )
        tok_sb = ctx.enter_context(
            nc.sbuf_tensor("tok_sb", [P, B * D], mybir.dt.float32)
        )
        s_ids = ctx.enter_context(nc.semaphore("s_ids"))
        s_pe = ctx.enter_context(nc.semaphore("s_pe"))
        s_a = ctx.enter_context(nc.semaphore("s_a"))
        # per-buffer-slot DMA completion sems (SDMA engines complete with skew)
        s_g = [ctx.enter_context(nc.semaphore(f"s_g{j}")) for j in range(B)]
        s_st = [ctx.enter_context(nc.semaphore(f"s_st{j}")) for j in range(B)]
        block = ctx.enter_context(nc.Block())

        @block.sync
        def _(sync):
            sync.dma_start(ids_sb[:], ids_t[:]).then_inc(s_ids, 16)
            sync.dma_start(pe_sb[:], pe_t[:]).then_inc(s_pe, 16)
            for i in range(N_TILES):
                j = i % B
                sync.wait_ge(s_a, i + 1)
                sync.dma_start(
                    out_t[i * P : (i + 1) * P, :], tok_sb[:, j * D : (j + 1) * D]
                ).then_inc(s_st[j], 16)
            for j in range(B):
                sync.wait_ge(s_st[j], 16 * (N_TILES // B))

        @block.gpsimd
        def _(g):
            g.wait_ge(s_ids, 16)
            for i in range(N_TILES):
                j = i % B
                if i >= B:
                    # tok slot reuse: store of iteration i-B must be done
                    g.wait_ge(s_st[j], 16 * (i // B))
                g.indirect_dma_start(
                    out=tok_sb[:, j * D : (j + 1) * D],
                    out_offset=None,
                    in_=table_t[:],
                    in_offset=bass.IndirectOffsetOnAxis(
                        ap=ids_sb[:, i : i + 1], axis=0
                    ),
                ).then_inc(s_g[j], 16)

        @block.vector
        def _(v):
            v.wait_ge(s_pe, 16)
            for i in range(N_TILES):
                j = i % B
                c = i % CHUNKS  # pe chunk for this tile
                v.wait_ge(s_g[j], 16 * (i // B + 1))
                v.tensor_add(
                    out=tok_sb[:, j * D : (j + 1) * D],
                    in0=tok_sb[:, j * D : (j + 1) * D],
                    in1=pe_sb[:, c * D : (c + 1) * D],
                ).then_inc(s_a, 1)

    return nc


def _get_nc():
    if "nc" not in _CACHED:
        _CACHED["nc"] = _build_nc()
    return _CACHED["nc"]


def _in_maps(input_ids, emb_table):
    ids = np.ascontiguousarray(np.asarray(input_ids)).astype(np.int32)
    table = np.ascontiguousarray(np.asarray(emb_table), dtype=np.float32)
    assert ids.shape == (BATCH, SEQ)
    assert table.shape == (VOCAB, D)

    if "pe" not in _CACHED:
        _CACHED["pe"] = _sinusoidal_table(SEQ, D)
    pe = _CACHED["pe"]

    in_maps = []
    for cj in range(N_CORES):
        # tile i: batch row i//CHUNKS, positions cj*512 + (i%CHUNKS)*128 + p
        ids_slice = ids[:, cj * POS_PER_CORE : (cj + 1) * POS_PER_CORE]
        # [BATCH, CHUNKS, P] -> ids_re[p, i] with i = b*CHUNKS + c
        ids_re = np.ascontiguousarray(
            ids_slice.reshape(BATCH * CHUNKS, P).T
        )
        pe_slice = pe[cj * POS_PER_CORE : (cj + 1) * POS_PER_CORE]  # [512, D]
        pe_re = np.ascontiguousarray(
            pe_slice.reshape(CHUNKS, P, D).transpose(1, 0, 2).reshape(P, CHUNKS * D)
        )
        in_maps.append({"ids": ids_re, "table": table, "pe": pe_re})
    return in_maps


def _unshard(results):
    # core cj, tile i, partition p -> out[b = i//CHUNKS,
    #                                     l = cj*512 + (i%CHUNKS)*128 + p]
    out = np.empty((BATCH, SEQ, D), np.float32)
    ofull = out.reshape(BATCH, N_CORES, CHUNKS, P, D)
    for cj in range(N_CORES):
        ofull[:, cj] = results[cj]["out"].reshape(BATCH, CHUNKS, P, D)
    return out


def kernel(input_ids, emb_table, encoder_seq_L):
    nc = _get_nc()
    in_maps = _in_maps(input_ids, emb_table)
    res = run_bass_kernel_spmd(nc, in_maps, core_ids=list(range(N_CORES)))
    return _unshard(res.results)


# revision 15
# speedup vs baseline: 1.7691x; 1.0259x over previous
"""Positional-embedding lookup kernel for TRN2 (8 NeuronCores).

out[b, l, :] = emb_table[input_ids[b, l], :] + pe[l, :]

Sharding: data-parallel over SEQUENCE. Core j handles positions
[j*512, (j+1)*512) of all 8 batch rows (4096 tokens). This keeps the
per-core gather/store traffic identical to batch sharding (16 MiB each)
but shrinks the positional-encoding read to a 2 MiB slice that stays
resident in SBUF and is reused by all 8 batch rows.

Per-core tile i (0..31): batch row i//4, position chunk i%4 within the
core's 512-position range. The 4 distinct PE chunks are loaded once.

Raw Bass (not Tile): the per-instruction sync-wait slots on this compiler
are too few for the gather pipeline, so semaphores are managed manually
with standalone wait_ge instructions. DMA completion sems are
per-buffer-slot: same-slot DMAs are serialized by the pipeline, which
makes per-slot counting sound despite cross-SDMA-engine completion skew.
"""

import contextlib

import numpy as np

import concourse.bass as bass
import concourse.mybir as mybir
from concourse.bass_utils import run_bass_kernel_spmd

VOCAB = 50257
D = 1024
BATCH = 8
SEQ = 4096
P = 128
N_CORES = 8
POS_PER_CORE = SEQ // N_CORES  # 512
CHUNKS = POS_PER_CORE // P  # 4 pe chunks per core
N_TILES = BATCH * CHUNKS  # 32 tiles of 128 tokens per core
B = 8  # tok ring buffers

_CACHED = {}


def _sinusoidal_table(seq_len: int, d_model: int) -> np.ndarray:
    i = np.arange(0, d_model // 2, dtype=np.float32)
    pos = np.arange(seq_len, dtype=np.float32)[:, None]
    div = pos / np.power(np.float32(10000.0), 2.0 * i / np.float32(d_model))
    pe = np.stack((np.sin(div), np.cos(div)), axis=2).reshape(seq_len, -1)
    return np.ascontiguousarray(pe[:, :d_model], dtype=np.float32)


def _build_nc():
    nc = bass.Bass("TRN2")
    ids_t = nc.dram_tensor("ids", [P, N_TILES], mybir.dt.int32, kind="ExternalInput")
    table_t = nc.dram_tensor(
        "table", [VOCAB, D], mybir.dt.float32, kind="ExternalInput"
    )
    pe_t = nc.dram_tensor("pe", [P, CHUNKS * D], mybir.dt.float32, kind="ExternalInput")
    out_t = nc.dram_tensor("out", [N_TILES * P, D], mybir.dt.float32, kind="ExternalOutput")

    with contextlib.ExitStack() as ctx:
        ids_sb = ctx.enter_context(
            nc.sbuf_tensor("ids_sb", [P, N_TILES], mybir.dt.int32)
        )
        pe_sb = ctx.enter_context(
            nc.sbuf_tensor("pe_sb", [P, CHUNKS * D], mybir.dt.float32)
        )
        tok_sb = ctx.enter_context(
            nc.sbuf_tensor("tok_sb", [P, B * D], mybir.dt.float32)
        )
        # alloc without release: skips the per-sem clear chain at kernel end;
        # the prologue's range sem_clear re-zeroes them on every execution
        s_ids = nc.alloc_semaphore("s_ids")
        s_pe = nc.alloc_semaphore("s_pe")
        s_a = nc.alloc_semaphore("s_a")
        # per-buffer-slot DMA completion sems (SDMA engines complete with skew)
        s_g = [nc.alloc_semaphore(f"s_g{j}") for j in range(B)]
        s_st = [nc.alloc_semaphore(f"s_st{j}") for j in range(B)]
        block = ctx.enter_context(nc.Block(no_gpsimd_drain=True))

        @block.sync
        def _(sync):
            sync.dma_start(ids_sb[:], ids_t[:]).then_inc(s_ids, 16)
            sync.dma_start(pe_sb[:], pe_t[:]).then_inc(s_pe, 16)
            for i in range(N_TILES):
                j = i % B
                sync.wait_ge(s_a, i + 1)
                sync.dma_start(
                    out_t[i * P : (i + 1) * P, :], tok_sb[:, j * D : (j + 1) * D]
                ).then_inc(s_st[j], 16)
            for j in range(B):
                sync.wait_ge(s_st[j], 16 * (N_TILES // B))

        @block.gpsimd
        def _(g):
            g.wait_ge(s_ids, 16)
            for i in range(N_TILES):
                j = i % B
                if i >= B:
                    # tok slot reuse: store of iteration i-B must be done
                    g.wait_ge(s_st[j], 16 * (i // B))
                g.indirect_dma_start(
                    out=tok_sb[:, j * D : (j + 1) * D],
                    out_offset=None,
                    in_=table_t[:],
                    in_offset=bass.IndirectOffsetOnAxis(
                        ap=ids_sb[:, i : i + 1], axis=0
                    ),
                ).then_inc(s_g[j], 16)

        @block.vector
        def _(v):
            v.wait_ge(s_pe, 16)
            for i in range(N_TILES):
                j = i % B
                c = i % CHUNKS  # pe chunk for this tile
                v.wait_ge(s_g[j], 16 * (i // B + 1))
                v.tensor_add(
                    out=tok_sb[:, j * D : (j + 1) * D],
                    in0=tok_sb[:, j * D : (j + 1) * D],
                    in1=pe_sb[:, c * D : (c + 1) * D],
                ).then_inc(s_a, 1)

    return nc


def _get_nc():
    if "nc" not in _CACHED:
        _CACHED["nc"] = _build_nc()
    return _CACHED["nc"]


def _in_maps(input_ids, emb_table):
    ids = np.ascontiguousarray(np.asarray(input_ids)).astype(np.int32)
    table = np.ascontiguousarray(np.asarray(emb_table), dtype=np.float32)
    assert ids.shape == (BATCH, SEQ)
    assert table.shape == (VOCAB, D)

    if "pe" not in _CACHED:
        _CACHED["pe"] = _sinusoidal_table(SEQ, D)
    pe = _CACHED["pe"]

    in_maps = []
    for cj in range(N_CORES):
        # tile i: batch row i//CHUNKS, positions cj*512 + (i%CHUNKS)*128 + p
        ids_slice = ids[:, cj * POS_PER_CORE : (cj + 1) * POS_PER_CORE]
        # [BATCH, CHUNKS, P] -> ids_re[p, i] with i = b*CHUNKS + c
        ids_re = np.ascontiguousarray(
            ids_slice.reshape(BATCH * CHUNKS, P).T
        )
        pe_slice = pe[cj * POS_PER_CORE : (cj + 1) * POS_PER_CORE]  # [512, D]
        pe_re = np.ascontiguousarray(
            pe_slice.reshape(CHUNKS, P, D).transpose(1, 0, 2).reshape(P, CHUNKS * D)
        )
        in_maps.append({"ids": ids_re, "table": table, "pe": pe_re})
    return in_maps


def _unshard(results):
    # core cj, tile i, partition p -> out[b = i//CHUNKS,
    #                                     l = cj*512 + (i%CHUNKS)*128 + p]
    out = np.empty((BATCH, SEQ, D), np.float32)
    ofull = out.reshape(BATCH, N_CORES, CHUNKS, P, D)
    for cj in range(N_CORES):
        ofull[:, cj] = results[cj]["out"].reshape(BATCH, CHUNKS, P, D)
    return out


def kernel(input_ids, emb_table, encoder_seq_L):
    nc = _get_nc()
    in_maps = _in_maps(input_ids, emb_table)
    res = run_bass_kernel_spmd(nc, in_maps, core_ids=list(range(N_CORES)))
    return _unshard(res.results)
